# revision 42
# baseline (speedup 1.0000x reference)
"""Stereo cost volume on 8 Trainium2 NeuronCores (batch-parallel SPMD).

out[b,h,w,d] = sum_c ref[b,h,w+63-d,c] * aux[b,h,w,c]
  B=8, H=192, W=384, C=128, D=64, ref width 447.

Strategy:
  * Shard batch across the 8 cores (1 batch each); pure SPMD, no collectives.
  * Host pre-transposes inputs to [C, H, W] and quantizes to fp8 E3M4
    (float8e3, 4 mantissa bits) with scale 2.83: halves input DRAM traffic
    vs fp16 at rel err 1.60e-2 (verified exactly offline vs the 2e-2 gate;
    e4m3 would land at 3.8e-2 and fail).  The channel contraction (C=128)
    lands on SBUF partitions and feeds the 128x128 PE array exactly.
  * Per h-row, per 128-wide W chunk: 4 col-tiled matmuls (M=32 output
    positions each, tile_position=(0,32g)) stream a 95-column ref window
    into one PSUM tile laid out [128, 3*96].  Grouping output w-positions
    by 32 bounds each group's diagonal band inside 95 uniform columns.
    The pace-setter here is the weight path: each matmul's 32-col
    LDWEIGHTS serializes on the single weight XBUS (~107ns per 4-group
    chunk); a shared full-array LDWEIGHTS was tried and is NOT honored by
    the NEFF lowering (InstMatmult.ldweights=False still emits per-MM
    loads and the full-array load drains the strip pipeline: 1.5x SLOWER).
  * PSUM->SBUF eviction alternates DVE/ACT 1:1 (both copy streams run
    concurrently; eviction would otherwise pace the pipeline), casting to
    fp16 and dropping the 96th pad column (285 cols staged per h).
  * DENSE_OUT (disabled, see comment at the constant): gpsimd
    indirect_copy cannot compact the band 285 -> 192 cols; the shipped
    band keeps 95 cols per 32-row group (67% useful), which is the floor
    for uniform (non-per-partition) access patterns.
  * Large contiguous DMAs in (sync queue) and out (ACT queue); taper the
    first/last h-blocks so the pipeline fills and drains faster.
  * Host extraction is a zero-copy strided view + f32 upcast + unscale.

History: fp16 baseline 172us (55MB DRAM/core, DMA-bound at ~26.6 GB/s per
SDMA engine) -> fp8 inputs 130us -> eviction split + 285-col ship + head
taper + HAM warm-up burst 124-127us.  Rel err 1.6027e-2 (gate: 2e-2).
Dead ends, all measured: For_i loop 206us (loop control serializes);
standalone ldweights + InstMatmult.ldweights=False 200us (flag ignored by
the NEFF lowering, full-array LDW drains the strip pipeline); gpsimd
indirect_copy band compaction 1.04ms AND wrong (indices are per-core, not
per-partition); sustained per-h warmth pins K=8/8 but doesn't speed the
weight path.  DMA floor at 34.6MB is ~82us/engine-busy + ~12us edges.
"""

import sys

import ml_dtypes
import numpy as np

sys.path.insert(0, "/opt/trn_rl_repo")

import concourse.bass as bass
import concourse.mybir as mybir
from concourse import bacc, bass_utils
from concourse.tile import TileContext

B, H, W, C, D = 8, 192, 384, 128, 64
OFF = 63
REF_W = W + OFF  # 447
NCHUNK = W // 128  # 3
GW = 32  # output w-positions per col group
NGROUP = 128 // GW  # 4
WIN = GW + OFF  # 95 streamed ref columns per group
PSUM_BLK = 96  # column stride per chunk block in PSUM (bank-friendly pad)
BLK = 95  # column stride per chunk in the staged/shipped output (pad dropped)
OUT_COLS = NCHUNK * BLK  # 285
DCOLS = NCHUNK * D  # 192 dense output cols per h (band compacted on-device)
# gpsimd indirect_copy CANNOT extract the diagonal band: its index lists are
# per-core (wrapped across each 16-partition group), not per-partition, and the
# measured gather throughput (~1ms for 9.4MB) is ~8x too slow regardless
DENSE_OUT = False
HB = 12  # max h rows per input DMA block
OB = 16  # h rows per output staging buffer

F16 = mybir.dt.float16
F32 = mybir.dt.float32
F8 = mybir.dt.float8e3  # E3M4: 4 mantissa bits; halves input DRAM traffic
E3M4 = ml_dtypes.float8_e3m4
F8_MAX = 15.5
# inputs are N(0,1); scaling before the e3m4 cast trades subnormal truncation
# (small |x|) against clipping (|x| > 15.5/scale = 5.5 sigma, ~4e-8 of mass)
F8_SCALE = 2.8284271

# hardware For_i over the middle blocks shrinks the unrolled PE instruction
# stream (less IRAM fetch traffic, which rides the critical DMA engine)
USE_LOOP = False  # measured 206us vs 124us unrolled: loop control serializes
# issue a ~6us burst of dummy matmuls during the first input-DMA wait: the HAM
# clock gate only lifts (1.2 -> 2.4 GHz) after ~3.4us of sustained PE activity,
# and the real stream's duty cycle is too low to ever trip it on its own
WARMUP_MMS = 16
# a dummy matmul per h-row DOES pin the HAM warm (throttle 58us -> 26us) but
# the h-period got WORSE (444ns): the weight-load bus is fixed-clock and is the
# PE-side pacer, so sustained warmth buys nothing and the dummy adds bus time
SUSTAIN_MM = False
SUSTAIN_N = 384


def _build() -> bass.Bass:
    nc = bacc.Bacc("TRN2", target_bir_lowering=False, debug=False)
    ref_d = nc.dram_tensor("ref_t", [C, H, REF_W], F8, kind="ExternalInput").ap()
    aux_d = nc.dram_tensor("aux_t", [C, H, W], F8, kind="ExternalInput").ap()
    # output ships as fp16: the PSUM->SBUF staging copy casts for free and it
    # halves output DRAM traffic; adds ~1e-4 relative error on top of the
    # fp16-input error (~2.5e-4)
    ship_cols = DCOLS if DENSE_OUT else OUT_COLS
    out_d = nc.dram_tensor("out_raw", [128, H, ship_cols], F16, kind="ExternalOutput").ap()
    if DENSE_OUT:
        idx_d = nc.dram_tensor(
            "idx_t", [128, OB * DCOLS], mybir.dt.uint16, kind="ExternalInput"
        ).ap()

    with TileContext(nc) as tc:
        with (
            tc.tile_pool(name="inp", bufs=3) as inp,
            tc.tile_pool(name="outp", bufs=3) as outp,
            tc.tile_pool(name="dns", bufs=3) as dns,
            tc.tile_pool(name="idxp", bufs=1) as idxp,
            tc.tile_pool(name="ps", bufs=6, space="PSUM") as ps,
            tc.tile_pool(name="wps", bufs=1, space="PSUM") as wps,
        ):
            if DENSE_OUT:
                idx_sb = idxp.tile([128, OB * DCOLS], mybir.dt.uint16, name="idx_sb")
                nc.sync.dma_start(out=idx_sb, in_=idx_d)
            warm_sb = warm_ps = None
            if WARMUP_MMS or SUSTAIN_MM:
                warm_sb = idxp.tile([C, 512], F8, name="warm_sb")
                warm_ps = wps.tile([128, 512], F32, name="warm_ps")
                nc.vector.memset(warm_sb, 0)
            if WARMUP_MMS:
                # runs while the first input DMA is in flight (PE is idle then);
                # ~6us of back-to-back matmuls lifts the HAM clock gate before
                # the real stream starts
                for _ in range(WARMUP_MMS):
                    nc.tensor.matmul(
                        out=warm_ps,
                        lhsT=warm_sb[:, :128],
                        rhs=warm_sb,
                        start=True,
                        stop=True,
                    )
            def emit_block(hb, nh):
                """One h-block: load inputs, matmul+copy per h, store outputs.

                hb may be a python int or a symbolic loop variable; DRAM APs
                use ds() so both lower correctly.
                """
                ref_sb = inp.tile([C, HB * REF_W], F8, tag="ref", name="ref_sb")
                aux_sb = inp.tile([C, HB * W], F8, tag="aux", name="aux_sb")
                nc.sync.dma_start(
                    out=ref_sb[:, : nh * REF_W], in_=ref_d[:, bass.ds(hb, nh), :]
                )
                nc.sync.dma_start(
                    out=aux_sb[:, : nh * W], in_=aux_d[:, bass.ds(hb, nh), :]
                )
                for sub in range(0, nh, OB):
                    nsub = min(OB, nh - sub)
                    out_sb = outp.tile([128, OB * OUT_COLS], F16, tag="out", name="out_sb")
                    for hs in range(nsub):
                        hl = sub + hs
                        pt = ps.tile([128, NCHUNK * PSUM_BLK], F32, name="pt")
                        for k in range(NCHUNK):
                            for g in range(NGROUP):
                                w0 = 128 * k + GW * g
                                nc.tensor.matmul(
                                    out=pt[
                                        GW * g : GW * g + GW,
                                        PSUM_BLK * k : PSUM_BLK * k + WIN,
                                    ],
                                    lhsT=aux_sb[:, hl * W + w0 : hl * W + w0 + GW],
                                    rhs=ref_sb[:, hl * REF_W + w0 : hl * REF_W + w0 + WIN],
                                    start=True,
                                    stop=True,
                                    tile_position=(0, GW * g),
                                )
                        if SUSTAIN_MM:
                            nc.tensor.matmul(
                                out=warm_ps[0:32, 0:SUSTAIN_N],
                                lhsT=warm_sb[:, :32],
                                rhs=warm_sb[:, :SUSTAIN_N],
                                start=True,
                                stop=True,
                                tile_position=(0, 0),
                            )
                        # eviction paces the whole pipeline: split 1:1 across
                        # DVE and ACT so both copy streams run concurrently
                        copy_eng = (
                            nc.scalar.copy if hs % 2 == 1 else nc.vector.tensor_copy
                        )
                        copy_eng(
                            out=out_sb[
                                :, hs * OUT_COLS : (hs + 1) * OUT_COLS
                            ].rearrange("p (c b) -> p c b", c=NCHUNK),
                            in_=pt.rearrange("p (c b) -> p c b", c=NCHUNK)[:, :, :BLK],
                        )
                    if DENSE_OUT:
                        dense_sb = dns.tile([128, OB * DCOLS], F16, tag="dns", name="dense_sb")
                        # ISA limit: IndirectCopy dst element count <= 1024, so
                        # gather at most 4 h-rows (768 dst elems) per instruction
                        for o in range(0, nsub, 4):
                            n2 = min(4, nsub - o)
                            nc.gpsimd.indirect_copy(
                                out=dense_sb[:, o * DCOLS : (o + n2) * DCOLS],
                                data=out_sb[:, o * OUT_COLS : (o + n2) * OUT_COLS],
                                idxs=idx_sb[:, : n2 * DCOLS],
                                i_know_ap_gather_is_preferred=True,
                            )
                        store_sb, ncols = dense_sb, DCOLS
                    else:
                        store_sb, ncols = out_sb, OUT_COLS
                    # outputs go out on the Activation HWDGE queue so they don't
                    # serialize behind input loads on the sync queue
                    nc.scalar.dma_start(
                        out=out_d[:, bass.ds(hb + sub, nsub), :],
                        in_=store_sb[:, : nsub * ncols],
                    )

            # taper block sizes: small first blocks get the pipeline rolling
            # sooner; small last blocks shrink the compute+store drain tail
            head = [4, 8, 12]
            n_mid = 12
            tail = [8, 8, 4, 2, 2]
            assert sum(head) + n_mid * HB + sum(tail) == H
            hb = 0
            for nh in head:
                emit_block(hb, nh)
                hb += nh
            if USE_LOOP:
                with tc.For_i(
                    hb,
                    hb + n_mid * HB,
                    HB,
                    staggered_reset=True,
                    hint_engines=(mybir.EngineType.PE,),
                ) as hoff:
                    emit_block(hoff, HB)
            else:
                for _ in range(n_mid):
                    emit_block(hb, HB)
                    hb += HB
            hb = sum(head) + n_mid * HB
            for nh in tail:
                emit_block(hb, nh)
                hb += nh
    nc.compile()
    return nc


def _extract(core_out: np.ndarray) -> np.ndarray:
    """[128, H, 285] fp16 device output -> [H, W, D] f32 cost volume (one batch).

    Device row m = 32g + r, column 95k + c holds
    dot(aux[128k + m], ref[128k + 32g + c]); the band entry for
    w = 128k + m, disparity d sits at c = r + 63 - d.
    """
    sm, sh, sc = core_out.strides
    base = core_out[:, :, OFF:]
    v = np.lib.stride_tricks.as_strided(
        base,
        shape=(H, NCHUNK, NGROUP, GW, D),
        strides=(sh, BLK * sc, GW * sm, sm + sc, -sc),
    )
    out = np.ascontiguousarray(v).astype(np.float32).reshape(H, W, D)
    out *= 1.0 / (F8_SCALE * F8_SCALE)
    return out


def _extract_dense(core_out: np.ndarray) -> np.ndarray:
    """[128, H, 192] fp16 dense device output -> [H, W, D] f32 (one batch).

    Dense cell [p, h, 64k + d] holds dot(aux[128k + p], ref[128k + p + 63 - d]).
    """
    v = core_out.reshape(128, H, NCHUNK, D).transpose(1, 2, 0, 3)
    out = np.ascontiguousarray(v).astype(np.float32).reshape(H, W, D)
    out *= 1.0 / (F8_SCALE * F8_SCALE)
    return out


def _make_idx() -> np.ndarray:
    """Band-gather index table: idx[p, hs*192 + 64k + d] = hs*285 + 95k + p%32 + 63 - d."""
    p = np.arange(128)[:, None, None, None]
    hs = np.arange(OB)[None, :, None, None]
    k = np.arange(NCHUNK)[None, None, :, None]
    d = np.arange(D)[None, None, None, :]
    idx = hs * OUT_COLS + BLK * k + (p % GW) + OFF - d
    return np.ascontiguousarray(idx.reshape(128, OB * DCOLS).astype(np.uint16))


LAST_RESULTS = None


def _quant8(x: np.ndarray) -> np.ndarray:
    q = np.clip(x * F8_SCALE, -F8_MAX, F8_MAX).astype(E3M4)
    return np.ascontiguousarray(q.transpose(0, 3, 1, 2))


def kernel(ref: np.ndarray, aux: np.ndarray, _trace: bool = False) -> np.ndarray:
    global LAST_RESULTS
    ref16 = _quant8(ref)
    aux16 = _quant8(aux)
    nc = _build()
    in_maps = [{"ref_t": ref16[b], "aux_t": aux16[b]} for b in range(B)]
    if DENSE_OUT:
        idx = _make_idx()
        for m in in_maps:
            m["idx_t"] = idx
    res = bass_utils.run_bass_kernel_spmd(nc, in_maps, list(range(B)), trace=_trace)
    LAST_RESULTS = res
    ext = _extract_dense if DENSE_OUT else _extract
    return np.stack([ext(res.results[b]["out_raw"]) for b in range(B)], axis=0)

